# revision 27
# baseline (speedup 1.0000x reference)
"""BitLinear (ternary-weight + 8-bit-activation quantized matmul) on 8 TRN2 cores.

Strategy: data-parallel over tokens. Each core gets 2048 of the 16384 tokens
plus the full weight matrix, computes the whole BitLinear forward for its
token shard on device, and the host concatenates the shards.

Math (must match the jax reference):
  w_scale = max(mean(|W|), 1e-6)                       (scalar)
  w_q     = clip(round(W / w_scale), -1, 1)            (ternary)
  a       = clip(max_i |x|, 1e-8, inf)                 (per token)
  x_q     = clip(round(x * 127 / a), -127, 127)        (8-bit ints)
  y       = (x_q @ w_q^T) * w_scale * a / 127

All rounding is done with the fp32 magic-number trick (v + 1.5*2^23 - 1.5*2^23
is round-to-nearest-even), so device results bit-match jnp.round. x_q (ints
<= 127) and w_q ({-1,0,1}) are exact in bf16 and products accumulate exactly
in fp32 PSUM (|y_int| < 2^24), so the bf16 TensorE matmul is exact.
"""

from contextlib import ExitStack

import numpy as np

import concourse.bass as bass
import concourse.tile as tile
from concourse import bacc, bass_isa, mybir
from concourse.bass import ds, ts
from concourse.bass_utils import run_bass_kernel_spmd

F32 = mybir.dt.float32
BF16 = mybir.dt.bfloat16
AF = mybir.ActivationFunctionType
OP = mybir.AluOpType
AX = mybir.AxisListType

B, S, D_IN, D_OUT = 4, 4096, 2048, 2048
N_CORES = 8
TOK = B * S                # 16384 tokens
TPC = TOK // N_CORES       # 2048 tokens per core
NT = TPC // 128            # 16 token tiles per core
NJ = D_OUT // 128          # 16 weight row tiles
NI = D_IN // 128           # 16 contraction (k) blocks
NO = D_OUT // 512          # 4 output column blocks
CM = 12582912.0            # 1.5 * 2^23: fp32 RNE rounding magic
QMAX = 127.0

# Scheduling knobs (tuned via TimelineSim; see test notes).
KNOBS = {
    "ld_bufs": 4,
    "ldx_bufs": 2,
    "t1_bufs": 3,
    "inplace_t1": False,
    "abs_on_act": True,
    "shard_pass1": True,
    "wq_bufs": 4,
    "xqt_bufs": 4,
    "ys_bufs": 2,
    "pref": 4,
    "tpose_ring": "scalar",   # engine issuing DMA transposes
    "w2_batch": 4,            # pass-2: quantize N tiles, then N transposes
}

_CACHE = {}


def _emit(tc: tile.TileContext, x_d: bass.AP, w_d: bass.AP, ws_d: bass.AP, y_d: bass.AP):
    nc = tc.nc
    with ExitStack() as ctx:
        ld = ctx.enter_context(tc.tile_pool(name="ld", bufs=KNOBS["ld_bufs"]))
        ldx = ctx.enter_context(tc.tile_pool(name="ldx", bufs=KNOBS["ldx_bufs"]))
        t1p = (
            ctx.enter_context(tc.tile_pool(name="t1p", bufs=KNOBS["t1_bufs"]))
            if not KNOBS["inplace_t1"]
            else None
        )
        wqp = ctx.enter_context(tc.tile_pool(name="wqp", bufs=KNOBS["wq_bufs"]))
        xqp = ctx.enter_context(tc.tile_pool(name="xqp", bufs=2))
        xqtp = ctx.enter_context(tc.tile_pool(name="xqtp", bufs=KNOBS["xqt_bufs"]))
        wqtp = ctx.enter_context(tc.tile_pool(name="wqtp", bufs=1))
        ysp = ctx.enter_context(tc.tile_pool(name="ysp", bufs=KNOBS["ys_bufs"]))
        stats = ctx.enter_context(tc.tile_pool(name="stats", bufs=5))
        consts = ctx.enter_context(tc.tile_pool(name="consts", bufs=1))
        wsp = ctx.enter_context(tc.tile_pool(name="wsp", bufs=1))
        psum = ctx.enter_context(
            tc.tile_pool(name="psum", bufs=8, space=bass.MemorySpace.PSUM)
        )
        dram = ctx.enter_context(
            tc.tile_pool(name="dram", bufs=2, space=bass.MemorySpace.DRAM)
        )

        tpose_eng = nc.scalar if KNOBS["tpose_ring"] == "scalar" else nc.sync
        cpos = consts.tile([128, 1], F32, tag="cpos")
        nc.vector.memset(cpos, CM)

        # ---- W pass 1: abs-sum over the weight matrix ----
        # Sharded mode: each core reduces only its own 1/8 of the rows (a
        # separate per-core "ws" input) and the partial [128,1] sums are
        # AllReduce-added across the 8 cores via a DRAM bounce. Cuts the
        # serial pre-scale W read from 16.8 MB to 2.1 MB per core.
        # On the Scalar engine (Abs + accum_out row-sum) so the DVE is free
        # for the x-prep and pass-2 quantization that overlap this phase.
        czero = consts.tile([128, 1], F32, tag="czero")
        nc.vector.memset(czero, 0.0)
        npass1 = NJ // N_CORES if KNOBS["shard_pass1"] else NJ
        wsums = wsp.tile([128, npass1], F32, tag="wsums")
        for j in range(npass1):
            wt = ld.tile([128, D_IN], F32, tag="ld")
            src = ws_d if KNOBS["shard_pass1"] else w_d
            nc.sync.dma_start(wt, src[ts(j, 128), :])
            if KNOBS["abs_on_act"]:
                nc.scalar.activation(
                    wt, wt, AF.Abs, bias=czero, accum_out=wsums[:, ds(j, 1)]
                )
            else:
                nc.vector.reduce_sum(
                    wsums[:, ds(j, 1)], wt, axis=AX.X, apply_absolute_value=True
                )

        wsum_p = stats.tile([128, 1], F32, tag="wsp")
        if npass1 > 1:
            nc.vector.reduce_sum(wsum_p, wsums, axis=AX.X)
        else:
            nc.vector.tensor_copy(wsum_p, wsums)
        if KNOBS["shard_pass1"]:
            cin = dram.tile([128, 1], F32, tag="cin")
            cout = dram.tile([128, 1], F32, tag="cout")
            nc.scalar.dma_start(cin, wsum_p)
            nc.gpsimd.collective_compute(
                "AllReduce",
                OP.add,
                replica_groups=[list(range(N_CORES))],
                ins=[cin.opt()],
                outs=[cout.opt()],
            )
            wsum_x = stats.tile([128, 1], F32, tag="wsx")
            nc.scalar.dma_start(wsum_x, cout)
        else:
            wsum_x = wsum_p
        wsum_all = stats.tile([128, 1], F32, tag="wsa")
        nc.gpsimd.partition_all_reduce(wsum_all, wsum_x, 128, bass_isa.ReduceOp.add)
        # w_scale = max(sum / (O*I), 1e-6); long-lived -> consts pool
        wscale = consts.tile([128, 1], F32, tag="wscale")
        nc.vector.tensor_scalar(
            wscale, wsum_all, 1.0 / (D_OUT * D_IN), 1e-6, OP.mult, OP.max
        )
        # rws ~= 1/w_scale with one Newton refinement
        r0 = stats.tile([128, 1], F32, tag="wr0")
        nc.vector.reciprocal(r0, wscale)
        ntt = stats.tile([128, 1], F32, tag="wntt")
        nc.vector.tensor_mul(ntt, wscale, r0)
        nc.vector.tensor_scalar(ntt, ntt, -1.0, 2.0, OP.mult, OP.add)
        rws = consts.tile([128, 1], F32, tag="rws")
        nc.vector.tensor_mul(rws, r0, ntt)
        ws127 = consts.tile([128, 1], F32, tag="ws127")
        nc.vector.tensor_scalar(ws127, wscale, 1.0 / QMAX, None, OP.mult)

        # x-side prep chain: load, per-token scales, quantize, transpose.
        # Bulk loads ride the SWDGE (gpsimd) ring so the sync (HWDGE) ring
        # carries only DMA transposes - mixing them pays an xbar-mode drain
        # per switch.
        def x_prep(t):
            xt = ldx.tile([128, D_IN], F32, tag="ldx", name=f"xt{t}")
            nc.sync.dma_start(xt, x_d[ts(t, 128), :])
            a = stats.tile([128, 1], F32, tag="xa", name=f"xa{t}")
            nc.vector.reduce_max(a, xt, axis=AX.X, apply_absolute_value=True)
            nc.vector.tensor_scalar(a, a, 1e-8, None, OP.max)
            r0 = stats.tile([128, 1], F32, tag="xr0", name=f"xr0{t}")
            nc.vector.reciprocal(r0, a)
            ntt = stats.tile([128, 1], F32, tag="xntt", name=f"xntt{t}")
            nc.vector.tensor_mul(ntt, a, r0)
            nc.vector.tensor_scalar(ntt, ntt, -1.0, 2.0, OP.mult, OP.add)
            s = stats.tile([128, 1], F32, tag="xs", name=f"xs{t}")
            nc.vector.tensor_mul(s, r0, ntt)
            nc.vector.tensor_scalar(s, s, QMAX, None, OP.mult)  # 127/a
            sout = stats.tile([128, 1], F32, tag="xsout", name=f"xsout{t}")
            nc.vector.tensor_mul(sout, a, ws127)  # a * w_scale / 127

            if KNOBS["inplace_t1"]:
                t1 = xt
            else:
                t1 = t1p.tile([128, D_IN], F32, tag="t1", name=f"xt1_{t}")
            nc.scalar.activation(t1, xt, AF.Identity, bias=cpos, scale=s)
            xq = xqp.tile([128, D_IN], BF16, tag="xq", name=f"xq{t}")
            nc.vector.tensor_scalar(xq, t1, -CM, None, OP.add)
            # one-shot transpose: xqT[p, b, t'] = xq[t', b*128+p]
            xqT = xqtp.tile([128, NI, 128], BF16, tag="xqT", name=f"xqT{t}")
            tpose_eng.dma_start(xqT, xq, transpose=True)
            return xqT, sout

        # Prefetch the first token tiles' quantization so the GEMM can start
        # the moment the first wqT quarter lands.
        PREF = KNOBS["pref"]
        prefetched = [x_prep(t) for t in range(PREF)]

        # ---- W pass 2: quantize + transpose ----
        # One quarter tile per 512-wide output block so the GEMM can start as
        # soon as its own four j-tiles are quantized (whole-tile deps would
        # otherwise stall the first matmul on the last wqT write).
        # wqT[no][i_in, jq, i_blk, o_in] = w_q[(no*4+jq)*128 + o_in, i_blk*128 + i_in]
        wqT = [
            wqtp.tile(
                [128, NJ // NO, NI, 128], BF16, tag=f"wqT{no}", name=f"wqT{no}"
            )
            for no in range(NO)
        ]
        WB = KNOBS["w2_batch"]
        for j0 in range(0, NJ, WB):
            wqs = []
            for j in range(j0, min(j0 + WB, NJ)):
                wt = ld.tile([128, D_IN], F32, tag="ld", name=f"wt2_{j}")
                nc.sync.dma_start(wt, w_d[ts(j, 128), :])
                if KNOBS["inplace_t1"]:
                    t1 = wt
                else:
                    t1 = t1p.tile([128, D_IN], F32, tag="t1", name=f"wt1_{j}")
                # t1 = W * rws + CM  (fp32 add at ulp=1 == RNE round)
                nc.scalar.activation(t1, wt, AF.Identity, bias=cpos, scale=rws)
                # clip in the offset domain: min(max(t1, CM-1), CM+1)
                nc.vector.tensor_scalar(
                    t1, t1, CM - 1.0, CM + 1.0, OP.max, OP.min
                )
                wq = wqp.tile([128, D_IN], BF16, tag=f"wqn{j % KNOBS['wq_bufs']}", name=f"wq{j}", bufs=1)
                nc.vector.tensor_scalar(wq, t1, -CM, None, OP.add)
                wqs.append((j, wq))
            for j, wq in wqs:
                tpose_eng.dma_start(
                    wqT[j // 4][:, j % 4, :, :], wq, transpose=True
                )

        # ---- main loop over token tiles ----
        for t in range(NT):
            if t < PREF:
                xqT, sout = prefetched[t]
            else:
                xqT, sout = x_prep(t)

            ys = ysp.tile([128, D_OUT], F32, tag="ys")
            for no in range(NO):
                ps = psum.tile([128, 512], F32, tag="ps")
                for b in range(NI):
                    nc.tensor.matmul(
                        ps,
                        xqT[:, b, :],
                        wqT[no][:, :, b, :],
                        start=(b == 0),
                        stop=(b == NI - 1),
                    )
                nc.vector.tensor_scalar(
                    ys[:, ts(no, 512)], ps, sout, None, OP.mult
                )
            nc.sync.dma_start(y_d[ts(t, 128), :], ys)


def _build():
    key = tuple(sorted(KNOBS.items()))
    if key in _CACHE:
        return _CACHE[key]
    nc = bacc.Bacc(
        "TRN2", target_bir_lowering=False, debug=False, num_devices=N_CORES
    )
    x_d = nc.dram_tensor("x", [TPC, D_IN], F32, kind="ExternalInput").ap()
    w_d = nc.dram_tensor("w", [D_OUT, D_IN], F32, kind="ExternalInput").ap()
    ws_d = nc.dram_tensor(
        "ws", [D_OUT // N_CORES, D_IN], F32, kind="ExternalInput"
    ).ap()
    y_d = nc.dram_tensor("y", [TPC, D_OUT], F32, kind="ExternalOutput").ap()
    with tile.TileContext(nc) as tc:
        _emit(tc, x_d, w_d, ws_d, y_d)
    nc.compile()
    _CACHE[key] = nc
    return nc


_last_result = None  # BassKernelResults of the most recent run (for profiling)


def kernel(x: np.ndarray, weight: np.ndarray, trace: bool = False) -> np.ndarray:
    global _last_result
    nc = _build()
    xf = np.ascontiguousarray(x.reshape(TOK, D_IN), dtype=np.float32)
    wf = np.ascontiguousarray(weight, dtype=np.float32)
    osh = D_OUT // N_CORES
    in_maps = [
        {
            "x": xf[c * TPC:(c + 1) * TPC],
            "w": wf,
            "ws": wf[c * osh:(c + 1) * osh],
        }
        for c in range(N_CORES)
    ]
    res = run_bass_kernel_spmd(nc, in_maps, list(range(N_CORES)), trace=trace)
    _last_result = res
    y = np.concatenate([res.results[c]["y"] for c in range(N_CORES)], axis=0)
    return y.reshape(B, S, D_OUT)


# revision 28
# speedup vs baseline: 1.0149x; 1.0149x over previous
"""BitLinear (ternary-weight + 8-bit-activation quantized matmul) on 8 TRN2 cores.

Strategy: data-parallel over tokens. Each core gets 2048 of the 16384 tokens
plus the full weight matrix, computes the whole BitLinear forward for its
token shard on device, and the host concatenates the shards.

Math (must match the jax reference):
  w_scale = max(mean(|W|), 1e-6)                       (scalar)
  w_q     = clip(round(W / w_scale), -1, 1)            (ternary)
  a       = clip(max_i |x|, 1e-8, inf)                 (per token)
  x_q     = clip(round(x * 127 / a), -127, 127)        (8-bit ints)
  y       = (x_q @ w_q^T) * w_scale * a / 127

All rounding is done with the fp32 magic-number trick (v + 1.5*2^23 - 1.5*2^23
is round-to-nearest-even), so device results bit-match jnp.round. x_q (ints
<= 127) and w_q ({-1,0,1}) are exact in bf16 and products accumulate exactly
in fp32 PSUM (|y_int| < 2^24), so the bf16 TensorE matmul is exact.
"""

from contextlib import ExitStack

import numpy as np

import concourse.bass as bass
import concourse.tile as tile
from concourse import bacc, bass_isa, mybir
from concourse.bass import ds, ts
from concourse.bass_utils import run_bass_kernel_spmd

F32 = mybir.dt.float32
BF16 = mybir.dt.bfloat16
AF = mybir.ActivationFunctionType
OP = mybir.AluOpType
AX = mybir.AxisListType

B, S, D_IN, D_OUT = 4, 4096, 2048, 2048
N_CORES = 8
TOK = B * S                # 16384 tokens
TPC = TOK // N_CORES       # 2048 tokens per core
NT = TPC // 128            # 16 token tiles per core
NJ = D_OUT // 128          # 16 weight row tiles
NI = D_IN // 128           # 16 contraction (k) blocks
NO = D_OUT // 512          # 4 output column blocks
CM = 12582912.0            # 1.5 * 2^23: fp32 RNE rounding magic
QMAX = 127.0

# Scheduling knobs (tuned via TimelineSim; see test notes).
KNOBS = {
    "ld_bufs": 4,
    "ldx_bufs": 3,
    "t1_bufs": 3,
    "inplace_t1": False,
    "abs_on_act": True,
    "shard_pass1": False,
    "wq_bufs": 3,
    "xqt_bufs": 3,
    "ys_bufs": 3,
    "pref": 3,
    "tpose_ring": "scalar",   # engine issuing DMA transposes
    "w2_batch": 1,            # pass-2: quantize N tiles, then N transposes
}

_CACHE = {}


def _emit(tc: tile.TileContext, x_d: bass.AP, w_d: bass.AP, ws_d: bass.AP, y_d: bass.AP):
    nc = tc.nc
    with ExitStack() as ctx:
        ld = ctx.enter_context(tc.tile_pool(name="ld", bufs=KNOBS["ld_bufs"]))
        ldx = ctx.enter_context(tc.tile_pool(name="ldx", bufs=KNOBS["ldx_bufs"]))
        t1p = (
            ctx.enter_context(tc.tile_pool(name="t1p", bufs=KNOBS["t1_bufs"]))
            if not KNOBS["inplace_t1"]
            else None
        )
        wqp = ctx.enter_context(tc.tile_pool(name="wqp", bufs=KNOBS["wq_bufs"]))
        xqp = ctx.enter_context(tc.tile_pool(name="xqp", bufs=2))
        xqtp = ctx.enter_context(tc.tile_pool(name="xqtp", bufs=KNOBS["xqt_bufs"]))
        wqtp = ctx.enter_context(tc.tile_pool(name="wqtp", bufs=1))
        ysp = ctx.enter_context(tc.tile_pool(name="ysp", bufs=KNOBS["ys_bufs"]))
        stats = ctx.enter_context(tc.tile_pool(name="stats", bufs=5))
        consts = ctx.enter_context(tc.tile_pool(name="consts", bufs=1))
        wsp = ctx.enter_context(tc.tile_pool(name="wsp", bufs=1))
        psum = ctx.enter_context(
            tc.tile_pool(name="psum", bufs=8, space=bass.MemorySpace.PSUM)
        )
        dram = ctx.enter_context(
            tc.tile_pool(name="dram", bufs=2, space=bass.MemorySpace.DRAM)
        )

        tpose_eng = nc.scalar if KNOBS["tpose_ring"] == "scalar" else nc.sync
        cpos = consts.tile([128, 1], F32, tag="cpos")
        nc.vector.memset(cpos, CM)

        # ---- W pass 1: abs-sum over the weight matrix ----
        # Sharded mode: each core reduces only its own 1/8 of the rows (a
        # separate per-core "ws" input) and the partial [128,1] sums are
        # AllReduce-added across the 8 cores via a DRAM bounce. Cuts the
        # serial pre-scale W read from 16.8 MB to 2.1 MB per core.
        # On the Scalar engine (Abs + accum_out row-sum) so the DVE is free
        # for the x-prep and pass-2 quantization that overlap this phase.
        czero = consts.tile([128, 1], F32, tag="czero")
        nc.vector.memset(czero, 0.0)
        npass1 = NJ // N_CORES if KNOBS["shard_pass1"] else NJ
        wsums = wsp.tile([128, npass1], F32, tag="wsums")
        for j in range(npass1):
            wt = ld.tile([128, D_IN], F32, tag="ld")
            src = ws_d if KNOBS["shard_pass1"] else w_d
            nc.sync.dma_start(wt, src[ts(j, 128), :])
            if KNOBS["abs_on_act"]:
                nc.scalar.activation(
                    wt, wt, AF.Abs, bias=czero, accum_out=wsums[:, ds(j, 1)]
                )
            else:
                nc.vector.reduce_sum(
                    wsums[:, ds(j, 1)], wt, axis=AX.X, apply_absolute_value=True
                )

        wsum_p = stats.tile([128, 1], F32, tag="wsp")
        if npass1 > 1:
            nc.vector.reduce_sum(wsum_p, wsums, axis=AX.X)
        else:
            nc.vector.tensor_copy(wsum_p, wsums)
        if KNOBS["shard_pass1"]:
            cin = dram.tile([128, 1], F32, tag="cin")
            cout = dram.tile([128, 1], F32, tag="cout")
            nc.scalar.dma_start(cin, wsum_p)
            nc.gpsimd.collective_compute(
                "AllReduce",
                OP.add,
                replica_groups=[list(range(N_CORES))],
                ins=[cin.opt()],
                outs=[cout.opt()],
            )
            wsum_x = stats.tile([128, 1], F32, tag="wsx")
            nc.scalar.dma_start(wsum_x, cout)
        else:
            wsum_x = wsum_p
        wsum_all = stats.tile([128, 1], F32, tag="wsa")
        nc.gpsimd.partition_all_reduce(wsum_all, wsum_x, 128, bass_isa.ReduceOp.add)
        # w_scale = max(sum / (O*I), 1e-6); long-lived -> consts pool
        wscale = consts.tile([128, 1], F32, tag="wscale")
        nc.vector.tensor_scalar(
            wscale, wsum_all, 1.0 / (D_OUT * D_IN), 1e-6, OP.mult, OP.max
        )
        # rws ~= 1/w_scale with one Newton refinement
        r0 = stats.tile([128, 1], F32, tag="wr0")
        nc.vector.reciprocal(r0, wscale)
        ntt = stats.tile([128, 1], F32, tag="wntt")
        nc.vector.tensor_mul(ntt, wscale, r0)
        nc.vector.tensor_scalar(ntt, ntt, -1.0, 2.0, OP.mult, OP.add)
        rws = consts.tile([128, 1], F32, tag="rws")
        nc.vector.tensor_mul(rws, r0, ntt)
        ws127 = consts.tile([128, 1], F32, tag="ws127")
        nc.vector.tensor_scalar(ws127, wscale, 1.0 / QMAX, None, OP.mult)

        # x-side prep chain: load, per-token scales, quantize, transpose.
        # Bulk loads ride the SWDGE (gpsimd) ring so the sync (HWDGE) ring
        # carries only DMA transposes - mixing them pays an xbar-mode drain
        # per switch.
        def x_prep(t):
            xt = ldx.tile([128, D_IN], F32, tag="ldx", name=f"xt{t}")
            nc.sync.dma_start(xt, x_d[ts(t, 128), :])
            a = stats.tile([128, 1], F32, tag="xa", name=f"xa{t}")
            nc.vector.reduce_max(a, xt, axis=AX.X, apply_absolute_value=True)
            nc.vector.tensor_scalar(a, a, 1e-8, None, OP.max)
            r0 = stats.tile([128, 1], F32, tag="xr0", name=f"xr0{t}")
            nc.vector.reciprocal(r0, a)
            ntt = stats.tile([128, 1], F32, tag="xntt", name=f"xntt{t}")
            nc.vector.tensor_mul(ntt, a, r0)
            nc.vector.tensor_scalar(ntt, ntt, -1.0, 2.0, OP.mult, OP.add)
            s = stats.tile([128, 1], F32, tag="xs", name=f"xs{t}")
            nc.vector.tensor_mul(s, r0, ntt)
            nc.vector.tensor_scalar(s, s, QMAX, None, OP.mult)  # 127/a
            sout = stats.tile([128, 1], F32, tag="xsout", name=f"xsout{t}")
            nc.vector.tensor_mul(sout, a, ws127)  # a * w_scale / 127

            if KNOBS["inplace_t1"]:
                t1 = xt
            else:
                t1 = t1p.tile([128, D_IN], F32, tag="t1", name=f"xt1_{t}")
            nc.scalar.activation(t1, xt, AF.Identity, bias=cpos, scale=s)
            xq = xqp.tile([128, D_IN], BF16, tag="xq", name=f"xq{t}")
            nc.vector.tensor_scalar(xq, t1, -CM, None, OP.add)
            # one-shot transpose: xqT[p, b, t'] = xq[t', b*128+p]
            xqT = xqtp.tile([128, NI, 128], BF16, tag="xqT", name=f"xqT{t}")
            tpose_eng.dma_start(xqT, xq, transpose=True)
            return xqT, sout

        # Prefetch the first token tiles' quantization so the GEMM can start
        # the moment the first wqT quarter lands.
        PREF = KNOBS["pref"]
        prefetched = [x_prep(t) for t in range(PREF)]

        # ---- W pass 2: quantize + transpose ----
        # One quarter tile per 512-wide output block so the GEMM can start as
        # soon as its own four j-tiles are quantized (whole-tile deps would
        # otherwise stall the first matmul on the last wqT write).
        # wqT[no][i_in, jq, i_blk, o_in] = w_q[(no*4+jq)*128 + o_in, i_blk*128 + i_in]
        wqT = [
            wqtp.tile(
                [128, NJ // NO, NI, 128], BF16, tag=f"wqT{no}", name=f"wqT{no}"
            )
            for no in range(NO)
        ]
        WB = KNOBS["w2_batch"]
        for j0 in range(0, NJ, WB):
            wqs = []
            for j in range(j0, min(j0 + WB, NJ)):
                wt = ld.tile([128, D_IN], F32, tag="ld", name=f"wt2_{j}")
                nc.sync.dma_start(wt, w_d[ts(j, 128), :])
                if KNOBS["inplace_t1"]:
                    t1 = wt
                else:
                    t1 = t1p.tile([128, D_IN], F32, tag="t1", name=f"wt1_{j}")
                # t1 = W * rws + CM  (fp32 add at ulp=1 == RNE round)
                nc.scalar.activation(t1, wt, AF.Identity, bias=cpos, scale=rws)
                # clip in the offset domain: min(max(t1, CM-1), CM+1)
                nc.vector.tensor_scalar(
                    t1, t1, CM - 1.0, CM + 1.0, OP.max, OP.min
                )
                wq = wqp.tile([128, D_IN], BF16, tag=f"wqn{j % KNOBS['wq_bufs']}", name=f"wq{j}", bufs=1)
                nc.vector.tensor_scalar(wq, t1, -CM, None, OP.add)
                wqs.append((j, wq))
            for j, wq in wqs:
                tpose_eng.dma_start(
                    wqT[j // 4][:, j % 4, :, :], wq, transpose=True
                )

        # ---- main loop over token tiles ----
        for t in range(NT):
            if t < PREF:
                xqT, sout = prefetched[t]
            else:
                xqT, sout = x_prep(t)

            ys = ysp.tile([128, D_OUT], F32, tag="ys")
            for no in range(NO):
                ps = psum.tile([128, 512], F32, tag="ps")
                for b in range(NI):
                    nc.tensor.matmul(
                        ps,
                        xqT[:, b, :],
                        wqT[no][:, :, b, :],
                        start=(b == 0),
                        stop=(b == NI - 1),
                    )
                nc.vector.tensor_scalar(
                    ys[:, ts(no, 512)], ps, sout, None, OP.mult
                )
            nc.sync.dma_start(y_d[ts(t, 128), :], ys)


def _build():
    key = tuple(sorted(KNOBS.items()))
    if key in _CACHE:
        return _CACHE[key]
    nc = bacc.Bacc(
        "TRN2", target_bir_lowering=False, debug=False, num_devices=N_CORES
    )
    x_d = nc.dram_tensor("x", [TPC, D_IN], F32, kind="ExternalInput").ap()
    w_d = nc.dram_tensor("w", [D_OUT, D_IN], F32, kind="ExternalInput").ap()
    ws_d = nc.dram_tensor(
        "ws", [D_OUT // N_CORES, D_IN], F32, kind="ExternalInput"
    ).ap()
    y_d = nc.dram_tensor("y", [TPC, D_OUT], F32, kind="ExternalOutput").ap()
    with tile.TileContext(nc) as tc:
        _emit(tc, x_d, w_d, ws_d, y_d)
    nc.compile()
    _CACHE[key] = nc
    return nc


_last_result = None  # BassKernelResults of the most recent run (for profiling)


def kernel(x: np.ndarray, weight: np.ndarray, trace: bool = False) -> np.ndarray:
    global _last_result
    nc = _build()
    xf = np.ascontiguousarray(x.reshape(TOK, D_IN), dtype=np.float32)
    wf = np.ascontiguousarray(weight, dtype=np.float32)
    osh = D_OUT // N_CORES
    in_maps = [
        {
            "x": xf[c * TPC:(c + 1) * TPC],
            "w": wf,
            "ws": wf[c * osh:(c + 1) * osh],
        }
        for c in range(N_CORES)
    ]
    res = run_bass_kernel_spmd(nc, in_maps, list(range(N_CORES)), trace=trace)
    _last_result = res
    y = np.concatenate([res.results[c]["y"] for c in range(N_CORES)], axis=0)
    return y.reshape(B, S, D_OUT)
